# revision 3
# baseline (speedup 1.0000x reference)
"""Trainium2 Bass kernel for nn_Decoder (dense_cnn decoder head).

Sharding: 8 cores = 4 batches x 2 H-halves (batch is only 4, so each batch is
split into top/bottom 64 output rows; all halo logic is baked into host-side
per-core data so the SPMD program is uniform).

Device pipeline per core (all matmuls on the PE):
  stage1 : fused bilinear-upsample + per-pixel smooth conv, as one 128-K
           matmul per output row (host-built combined matrices) -> s1
           channel-major [2][128, 72*130+2]
  conv_a : 3x3 conv + folded BN + relu via 18 accumulating matmuls per
           512-pixel block over a flat padded-width layout -> a
  conv_b : same -> b2
  z      : commuted 1x1 conv (no bias) -> z pixel-major [128(w), 68*21]
  smooth2: banded per-row matmuls (host-built 5-diagonal matrices) + bias
           -> out [128(w), 64*21]
"""
import sys
import numpy as np

if '/opt/trn_rl_repo' not in sys.path:
    sys.path.insert(0, '/opt/trn_rl_repo')

EPS = 1e-5
B, C, HL, WL = 4, 256, 128, 128
HX = WX = 32
NCLS = 21
WP = 130                     # padded width; data cols 1..128
R_S1, R_A, R_B2, R_OUT = 72, 70, 68, 64
N_CORES = 8

# ---------------------------------------------------------------- host prep

def _interp_mat(n_out, n_in):
    s = np.linspace(0.0, n_in - 1.0, n_out)
    i0 = np.floor(s).astype(np.int64)
    f = s - i0
    i1 = np.minimum(i0 + 1, n_in - 1)
    M = np.zeros((n_out, n_in), np.float64)
    M[np.arange(n_out), i0] += 1.0 - f
    M[np.arange(n_out), i1] += f
    return M


_BY = _interp_mat(HL, HX)
_BX = _interp_mat(WL, WX)
_Y0 = np.floor(np.linspace(0.0, HX - 1.0, HL)).astype(np.int64)
_BXP = np.zeros((WL + 4, WX), np.float64)
_BXP[2:2 + WL] = _BX
_BXW = np.stack([_BXP[dj:dj + WL] for dj in range(5)], axis=1)   # [128, 5, 32]


def _core_ranges(core):
    return core // 2, 64 * (core % 2)


def _make_stage1(x_np, f4_np, core):
    b, r0 = _core_ranges(core)
    g = r0 - 4 + np.arange(R_S1)
    gv = (g >= 0) & (g < HL)
    gc = np.clip(g, 0, HL - 1)
    f4g = f4_np[b][gc] * gv[:, None, None, None]
    ybase = _Y0[np.clip(g - 2, 0, HL - 1)]
    T1 = np.einsum('rwij,wjx->rwix', f4g.astype(np.float64), _BXW)
    di = np.arange(5)
    r2 = g[:, None] + di[None, :] - 2
    rv = (r2 >= 0) & (r2 < HL)
    r2c = np.clip(r2, 0, HL - 1)
    t = np.arange(4)
    yidx = ybase[:, None] + t[None, :]
    ymask = yidx <= HX - 1
    yidxc = np.minimum(yidx, HX - 1)
    By_t = _BY[r2c[:, :, None], yidxc[:, None, :]] * rv[:, :, None] * ymask[:, None, :]
    S2 = np.einsum('rit,rwix->rtxw', By_t, T1).reshape(R_S1, 128, WL)
    rows = np.minimum(ybase[:, None] + t[None, :], HX - 1)
    xr = x_np[b][:, rows, :]
    xrep = np.ascontiguousarray(xr.transpose(1, 2, 3, 0).reshape(R_S1, 128, C))
    return xrep.astype(np.float32), S2.astype(np.float32)


def _make_sb(f4_np, core):
    b, r0 = _core_ranges(core)
    g2 = r0 + np.arange(R_OUT)
    Sb = np.zeros((R_OUT, 5, WL, WL), np.float32)
    di = np.arange(5)
    rv = ((g2[:, None] + di[None, :] - 2) >= 0) & \
         ((g2[:, None] + di[None, :] - 2) < HL)
    f4s = f4_np[b][g2]
    for dj in range(5):
        w = np.arange(max(0, 2 - dj), min(WL, WL + 2 - dj))
        wp = w + dj - 2
        Sb[:, :, wp, w] = (f4s[:, w, :, dj] * rv[None, :, :]
                           ).transpose(1, 2, 0).astype(np.float32)
    return Sb


def _fold_conv(w, gamma, beta, mean, var):
    inv = (np.asarray(gamma, np.float64)
           / np.sqrt(np.asarray(var, np.float64) + EPS))
    wf = np.asarray(w, np.float64) * inv[:, None, None, None]
    bias = np.asarray(beta, np.float64) - np.asarray(mean, np.float64) * inv
    t = wf.reshape(2, 128, 2, 128, 3, 3)
    t = t.transpose(2, 3, 0, 4, 5, 1)
    lhsT = np.ascontiguousarray(t.reshape(2, 128, 2 * 9 * 128)).astype(np.float32)
    return lhsT, bias.astype(np.float32)


def _make_masks(core):
    _, r0 = _core_ranges(core)
    top = np.array([1.0 if 0 <= (r0 - 3 + j) < HL else 0.0 for j in range(3)],
                   np.float32)
    bot = np.array([1.0 if 0 <= (r0 + 64 + j) < HL else 0.0 for j in range(3)],
                   np.float32)
    mt = np.broadcast_to(np.repeat(top, WP)[None, :], (128, 3 * WP)).copy()
    mb = np.broadcast_to(np.repeat(bot, WP)[None, :], (128, 3 * WP)).copy()
    return mt, mb


def _make_inmaps(inputs):
    x = np.asarray(inputs['x'], np.float32)
    f4 = np.asarray(inputs['filter4'], np.float32)
    wa_l, bias_a = _fold_conv(inputs['w_a'], inputs['gamma_a'],
                              inputs['beta_a'], inputs['mean_a'],
                              inputs['var_a'])
    wb_l, bias_b = _fold_conv(inputs['w_b'], inputs['gamma_b'],
                              inputs['beta_b'], inputs['mean_b'],
                              inputs['var_b'])
    bias_ab = np.stack([bias_a[:128], bias_a[128:],
                        bias_b[:128], bias_b[128:]], axis=1).astype(np.float32)
    wl = np.asarray(inputs['w_last'], np.float32)[:, :, 0, 0]
    wl_r = np.ascontiguousarray(wl.T.reshape(2, 128, NCLS)).astype(np.float32)
    wl_flat = np.concatenate([wl_r[0], wl_r[1]], axis=1)
    bias_l = np.broadcast_to(
        np.asarray(inputs['b_last'], np.float32)[None, :], (128, NCLS)).copy()
    maps = []
    for core in range(N_CORES):
        xrep, s2m = _make_stage1(x, f4, core)
        sbm = _make_sb(f4, core)
        mt, mb = _make_masks(core)
        maps.append(dict(xrep=xrep, s2m=s2m, sbm=sbm,
                         wa=wa_l, wb=wb_l, wl=wl_flat,
                         bias_ab=bias_ab, bias_l=bias_l,
                         mask_t=mt, mask_b=mb))
    return maps


# ---------------------------------------------------------------- device

_CACHE = {}


def _build():
    import concourse.bass as bass
    import concourse.bacc as bacc
    import concourse.mybir as mybir
    import concourse.tile as tile

    f32 = mybir.dt.float32
    Relu = mybir.ActivationFunctionType.Relu

    nc = bacc.Bacc("TRN2", target_bir_lowering=False, debug=False,
                   num_devices=N_CORES)

    d_xrep = nc.dram_tensor("xrep", [R_S1, 128, C], f32, kind="ExternalInput")
    d_s2m = nc.dram_tensor("s2m", [R_S1, 128, WL], f32, kind="ExternalInput")
    d_sbm = nc.dram_tensor("sbm", [R_OUT, 5, WL, WL], f32, kind="ExternalInput")
    d_wa = nc.dram_tensor("wa", [2, 128, 2304], f32, kind="ExternalInput")
    d_wb = nc.dram_tensor("wb", [2, 128, 2304], f32, kind="ExternalInput")
    d_wl = nc.dram_tensor("wl", [128, 2 * NCLS], f32, kind="ExternalInput")
    d_bab = nc.dram_tensor("bias_ab", [128, 4], f32, kind="ExternalInput")
    d_bl = nc.dram_tensor("bias_l", [128, NCLS], f32, kind="ExternalInput")
    d_mt = nc.dram_tensor("mask_t", [128, 3 * WP], f32, kind="ExternalInput")
    d_mb = nc.dram_tensor("mask_b", [128, 3 * WP], f32, kind="ExternalInput")
    d_out = nc.dram_tensor("out", [128, R_OUT, NCLS], f32, kind="ExternalOutput")

    S1_N, A_N, B2_N = R_S1 * WP + 2, R_A * WP + 2, R_B2 * WP + 2

    with tile.TileContext(nc) as tc:
        with (
            tc.tile_pool(name="wp", bufs=1) as wpool,
            tc.tile_pool(name="big", bufs=4) as bigpool,
            tc.tile_pool(name="xs", bufs=4) as xpool,
            tc.tile_pool(name="s2s", bufs=4) as s2pool,
            tc.tile_pool(name="sbs", bufs=4) as sbpool,
            tc.tile_pool(name="sm", bufs=1) as smpool,
            tc.tile_pool(name="ps", bufs=2, space="PSUM") as pp,
        ):
            # resident small tensors
            wa_t = [wpool.tile([128, 2304], f32, tag=f"wa{k}", name=f"wa{k}") for k in range(2)]
            wb_t = [wpool.tile([128, 2304], f32, tag=f"wb{k}", name=f"wb{k}") for k in range(2)]
            wl_t = wpool.tile([128, 2 * NCLS], f32, tag="wl", name="wl")
            bab_t = wpool.tile([128, 4], f32, tag="bab", name="bab")
            bl_t = wpool.tile([128, NCLS], f32, tag="bl", name="bl")
            mt_t = wpool.tile([128, 3 * WP], f32, tag="mt", name="mt")
            mb_t = wpool.tile([128, 3 * WP], f32, tag="mb", name="mb")
            for k in range(2):
                nc.sync.dma_start(wa_t[k][:], d_wa.ap()[k])
                nc.sync.dma_start(wb_t[k][:], d_wb.ap()[k])
            nc.sync.dma_start(wl_t[:], d_wl.ap())
            nc.sync.dma_start(bab_t[:], d_bab.ap())
            nc.sync.dma_start(bl_t[:], d_bl.ap())
            nc.sync.dma_start(mt_t[:], d_mt.ap())
            nc.sync.dma_start(mb_t[:], d_mb.ap())

            s1 = [bigpool.tile([128, S1_N], f32, tag="big", name=f"s1_{k}") for k in range(2)]
            a = [bigpool.tile([128, A_N], f32, tag="big", name=f"a_{k}") for k in range(2)]

            # zero s1 pad cols + boundary elems (never written by stage1)
            for k in range(2):
                v = s1[k][:, 1:1 + R_S1 * WP].rearrange(
                    "p (r w) -> p r w", w=WP)
                nc.vector.memset(v[:, :, 0:1], 0.0)
                nc.vector.memset(v[:, :, 129:130], 0.0)
                nc.vector.memset(s1[k][:, 0:1], 0.0)
                nc.vector.memset(s1[k][:, S1_N - 1:S1_N], 0.0)
                nc.vector.memset(a[k][:, 0:1], 0.0)
                nc.vector.memset(a[k][:, A_N - 1:A_N], 0.0)

            # ---- stage 1: fused upsample + smooth1
            for i in range(R_S1):
                xt = xpool.tile([128, C], f32, tag="xt", name="xt")
                nc.sync.dma_start(xt[:], d_xrep.ap()[i])
                st = s2pool.tile([128, WL], f32, tag="st", name="st")
                nc.sync.dma_start(st[:], d_s2m.ap()[i])
                for m in range(2):
                    ps = pp.tile([128, 128], f32, tag="s1p", name="s1p")
                    nc.tensor.matmul(ps[:], xt[:, m * 128:(m + 1) * 128],
                                     st[:], start=True, stop=True)
                    dst = s1[m][:, 1 + i * WP + 1:1 + i * WP + 129]
                    if (2 * i + m) % 2 == 0:
                        nc.scalar.copy(dst, ps[:])
                    else:
                        nc.vector.tensor_copy(dst, ps[:])

            # ---- conv helper
            def conv(inp, w_t, out_t, n_out, bias_col0):
                npix = n_out * WP
                nblk = (npix + 511) // 512
                for m in range(2):
                    for nb in range(nblk):
                        q0 = nb * 512
                        bs = min(512, npix - q0)
                        ps = pp.tile([128, 512], f32, tag="cp", name="cp")
                        idx = 0
                        for kt in range(2):
                            for di in range(3):
                                for dj in range(3):
                                    off = q0 + di * WP + dj
                                    nc.tensor.matmul(
                                        ps[:, :bs],
                                        w_t[kt][:, (m * 9 + di * 3 + dj) * 128:
                                                (m * 9 + di * 3 + dj) * 128 + 128],
                                        inp[kt][:, off:off + bs],
                                        start=(idx == 0), stop=(idx == 17))
                                    idx += 1
                        nc.scalar.activation(
                            out_t[m][:, 1 + q0:1 + q0 + bs], ps[:, :bs], Relu,
                            bias=bab_t[:, bias_col0 + m:bias_col0 + m + 1])

            # ---- conv_a, then mask halo rows + re-zero pad cols
            conv(s1, wa_t, a, R_A, 0)
            for m in range(2):
                nc.vector.tensor_mul(a[m][:, 1:1 + 3 * WP],
                                     a[m][:, 1:1 + 3 * WP], mt_t[:])
                o = 1 + (R_A - 3) * WP
                nc.vector.tensor_mul(a[m][:, o:o + 3 * WP],
                                     a[m][:, o:o + 3 * WP], mb_t[:])
                v = a[m][:, 1:1 + R_A * WP].rearrange("p (r w) -> p r w", w=WP)
                nc.vector.memset(v[:, :, 0:1], 0.0)
                nc.vector.memset(v[:, :, 129:130], 0.0)

            # ---- conv_b
            b2 = [bigpool.tile([128, B2_N], f32, tag="big", name=f"b2_{k}") for k in range(2)]
            conv(a, wb_t, b2, R_B2, 2)

            # ---- z = commuted 1x1 (pixel-major rows)
            z_pm = smpool.tile([128, R_B2 * NCLS], f32, tag="z", name="z_pm")
            for k in range(R_B2):
                ps = pp.tile([128, NCLS], f32, tag="zp", name="zp")
                for kt in range(2):
                    nc.tensor.matmul(ps[:],
                                     b2[kt][:, 1 + k * WP + 1:1 + k * WP + 129],
                                     wl_t[:, kt * NCLS:(kt + 1) * NCLS],
                                     start=(kt == 0), stop=(kt == 1))
                nc.vector.tensor_copy(z_pm[:, k * NCLS:(k + 1) * NCLS], ps[:])

            # ---- smooth2 (banded matmuls) + bias
            out_pm = smpool.tile([128, R_OUT * NCLS], f32, tag="o", name="out_pm")
            for m in range(R_OUT):
                ps = pp.tile([128, NCLS], f32, tag="op", name="op")
                for di in range(5):
                    sbt = sbpool.tile([128, WL], f32, tag="sbt", name="sbt")
                    nc.sync.dma_start(sbt[:], d_sbm.ap()[m][di])
                    nc.tensor.matmul(ps[:], sbt[:],
                                     z_pm[:, (m + di) * NCLS:(m + di + 1) * NCLS],
                                     start=(di == 0), stop=(di == 4))
                nc.vector.tensor_add(out_pm[:, m * NCLS:(m + 1) * NCLS],
                                     ps[:], bl_t[:])

            nc.sync.dma_start(d_out.ap(), out_pm[:])

    nc.compile()
    return nc


def kernel(**inputs):
    from concourse.bass_utils import run_bass_kernel_spmd

    if 'nc' not in _CACHE:
        _CACHE['nc'] = _build()
    nc = _CACHE['nc']

    maps = _make_inmaps(inputs)
    res = run_bass_kernel_spmd(nc, maps, list(range(N_CORES)), trace=False)

    out = np.zeros((B, NCLS, HL, WL), np.float32)
    for core in range(N_CORES):
        b, r0 = _core_ranges(core)
        out[b, :, r0:r0 + 64, :] = res.results[core]["out"].transpose(2, 1, 0)
    return out


# revision 4
# speedup vs baseline: 1.8898x; 1.8898x over previous
"""Trainium2 Bass kernel for nn_Decoder (dense_cnn decoder head).

Sharding: 8 cores = 4 batches x 2 H-halves (batch is only 4, so each batch is
split into top/bottom 64 output rows; all halo logic is baked into host-side
per-core data so the SPMD program is uniform).

Device pipeline per core (all matmuls on the PE, bf16 in / f32 PSUM):
  stage1 : fused bilinear-upsample + per-pixel smooth conv, as one 128-K
           matmul per output row (host-built combined matrices) -> s1
           channel-major [2][128, 72*130+2]
  conv_a : 3x3 conv + folded BN + relu via 18 accumulating matmuls per
           512-pixel block over a flat padded-width layout -> a
  conv_b : same -> b2
  z      : commuted 1x1 conv (no bias) -> z pixel-major [128(w), 68*21]
  smooth2: banded per-row matmuls (host-built 5-diagonal matrices) + bias
           -> out [128(w), 64*21] f32
"""
import sys
import numpy as np

if '/opt/trn_rl_repo' not in sys.path:
    sys.path.insert(0, '/opt/trn_rl_repo')

import ml_dtypes

BF16 = ml_dtypes.bfloat16

EPS = 1e-5
B, C, HL, WL = 4, 256, 128, 128
HX = WX = 32
NCLS = 21
WP = 130                     # padded width; data cols 1..128
R_S1, R_A, R_B2, R_OUT = 72, 70, 68, 64
N_CORES = 8

# ---------------------------------------------------------------- host prep

def _interp_mat(n_out, n_in):
    s = np.linspace(0.0, n_in - 1.0, n_out)
    i0 = np.floor(s).astype(np.int64)
    f = s - i0
    i1 = np.minimum(i0 + 1, n_in - 1)
    M = np.zeros((n_out, n_in), np.float64)
    M[np.arange(n_out), i0] += 1.0 - f
    M[np.arange(n_out), i1] += f
    return M


_BY = _interp_mat(HL, HX)
_BX = _interp_mat(WL, WX)
_Y0 = np.floor(np.linspace(0.0, HX - 1.0, HL)).astype(np.int64)
_BXP = np.zeros((WL + 4, WX), np.float64)
_BXP[2:2 + WL] = _BX
_BXW = np.stack([_BXP[dj:dj + WL] for dj in range(5)], axis=1)   # [128, 5, 32]


def _core_ranges(core):
    return core // 2, 64 * (core % 2)


def _make_stage1(x_np, f4_np, core):
    b, r0 = _core_ranges(core)
    g = r0 - 4 + np.arange(R_S1)
    gv = (g >= 0) & (g < HL)
    gc = np.clip(g, 0, HL - 1)
    f4g = f4_np[b][gc] * gv[:, None, None, None]
    ybase = _Y0[np.clip(g - 2, 0, HL - 1)]
    T1 = np.einsum('rwij,wjx->rwix', f4g.astype(np.float64), _BXW)
    di = np.arange(5)
    r2 = g[:, None] + di[None, :] - 2
    rv = (r2 >= 0) & (r2 < HL)
    r2c = np.clip(r2, 0, HL - 1)
    t = np.arange(4)
    yidx = ybase[:, None] + t[None, :]
    ymask = yidx <= HX - 1
    yidxc = np.minimum(yidx, HX - 1)
    By_t = _BY[r2c[:, :, None], yidxc[:, None, :]] * rv[:, :, None] * ymask[:, None, :]
    S2 = np.einsum('rit,rwix->rtxw', By_t, T1).reshape(R_S1, 128, WL)
    rows = np.minimum(ybase[:, None] + t[None, :], HX - 1)
    xr = x_np[b][:, rows, :]
    xrep = np.ascontiguousarray(xr.transpose(1, 2, 3, 0).reshape(R_S1, 128, C))
    return xrep.astype(BF16), S2.astype(BF16)


def _make_sb(f4_np, core):
    b, r0 = _core_ranges(core)
    g2 = r0 + np.arange(R_OUT)
    Sb = np.zeros((R_OUT, 5, WL, WL), BF16)
    di = np.arange(5)
    rv = ((g2[:, None] + di[None, :] - 2) >= 0) & \
         ((g2[:, None] + di[None, :] - 2) < HL)
    f4s = f4_np[b][g2]
    for dj in range(5):
        w = np.arange(max(0, 2 - dj), min(WL, WL + 2 - dj))
        wp = w + dj - 2
        Sb[:, :, wp, w] = (f4s[:, w, :, dj] * rv[None, :, :]
                           ).transpose(1, 2, 0).astype(BF16)
    return Sb


def _fold_conv(w, gamma, beta, mean, var):
    inv = (np.asarray(gamma, np.float64)
           / np.sqrt(np.asarray(var, np.float64) + EPS))
    wf = np.asarray(w, np.float64) * inv[:, None, None, None]
    bias = np.asarray(beta, np.float64) - np.asarray(mean, np.float64) * inv
    t = wf.reshape(2, 128, 2, 128, 3, 3)
    t = t.transpose(2, 3, 0, 4, 5, 1)
    lhsT = np.ascontiguousarray(t.reshape(2, 128, 2 * 9 * 128)).astype(BF16)
    return lhsT, bias.astype(np.float32)


def _make_masks(core):
    _, r0 = _core_ranges(core)
    top = np.array([1.0 if 0 <= (r0 - 3 + j) < HL else 0.0 for j in range(3)],
                   np.float32)
    bot = np.array([1.0 if 0 <= (r0 + 64 + j) < HL else 0.0 for j in range(3)],
                   np.float32)
    mt = np.broadcast_to(np.repeat(top, WP)[None, :], (128, 3 * WP))
    mb = np.broadcast_to(np.repeat(bot, WP)[None, :], (128, 3 * WP))
    return mt.astype(BF16), mb.astype(BF16)


def _make_inmaps(inputs):
    x = np.asarray(inputs['x'], np.float32)
    f4 = np.asarray(inputs['filter4'], np.float32)
    wa_l, bias_a = _fold_conv(inputs['w_a'], inputs['gamma_a'],
                              inputs['beta_a'], inputs['mean_a'],
                              inputs['var_a'])
    wb_l, bias_b = _fold_conv(inputs['w_b'], inputs['gamma_b'],
                              inputs['beta_b'], inputs['mean_b'],
                              inputs['var_b'])
    bias_ab = np.stack([bias_a[:128], bias_a[128:],
                        bias_b[:128], bias_b[128:]], axis=1).astype(np.float32)
    wl = np.asarray(inputs['w_last'], np.float32)[:, :, 0, 0]
    wl_r = np.ascontiguousarray(wl.T.reshape(2, 128, NCLS))
    wl_flat = np.concatenate([wl_r[0], wl_r[1]], axis=1).astype(BF16)
    bias_l = np.broadcast_to(
        np.asarray(inputs['b_last'], np.float32)[None, :], (128, NCLS)).copy()
    maps = []
    for core in range(N_CORES):
        xrep, s2m = _make_stage1(x, f4, core)
        sbm = _make_sb(f4, core)
        mt, mb = _make_masks(core)
        maps.append(dict(xrep=xrep, s2m=s2m, sbm=sbm,
                         wa=wa_l, wb=wb_l, wl=wl_flat,
                         bias_ab=bias_ab, bias_l=bias_l,
                         mask_t=mt, mask_b=mb))
    return maps


# ---------------------------------------------------------------- device

_CACHE = {}


def _build():
    import concourse.bacc as bacc
    import concourse.mybir as mybir
    import concourse.tile as tile

    f32 = mybir.dt.float32
    bf16 = mybir.dt.bfloat16
    Relu = mybir.ActivationFunctionType.Relu

    nc = bacc.Bacc("TRN2", target_bir_lowering=False, debug=False,
                   num_devices=N_CORES)

    d_xrep = nc.dram_tensor("xrep", [R_S1, 128, C], bf16, kind="ExternalInput")
    d_s2m = nc.dram_tensor("s2m", [R_S1, 128, WL], bf16, kind="ExternalInput")
    d_sbm = nc.dram_tensor("sbm", [R_OUT, 5, WL, WL], bf16, kind="ExternalInput")
    d_wa = nc.dram_tensor("wa", [2, 128, 2304], bf16, kind="ExternalInput")
    d_wb = nc.dram_tensor("wb", [2, 128, 2304], bf16, kind="ExternalInput")
    d_wl = nc.dram_tensor("wl", [128, 2 * NCLS], bf16, kind="ExternalInput")
    d_bab = nc.dram_tensor("bias_ab", [128, 4], f32, kind="ExternalInput")
    d_bl = nc.dram_tensor("bias_l", [128, NCLS], f32, kind="ExternalInput")
    d_mt = nc.dram_tensor("mask_t", [128, 3 * WP], bf16, kind="ExternalInput")
    d_mb = nc.dram_tensor("mask_b", [128, 3 * WP], bf16, kind="ExternalInput")
    d_out = nc.dram_tensor("out", [128, R_OUT, NCLS], f32, kind="ExternalOutput")

    S1_N, A_N, B2_N = R_S1 * WP + 2, R_A * WP + 2, R_B2 * WP + 2

    with tile.TileContext(nc) as tc:
        with (
            tc.tile_pool(name="wp", bufs=1) as wpool,
            tc.tile_pool(name="big", bufs=4) as bigpool,
            tc.tile_pool(name="xs", bufs=8) as xpool,
            tc.tile_pool(name="s2s", bufs=8) as s2pool,
            tc.tile_pool(name="sbs", bufs=8) as sbpool,
            tc.tile_pool(name="sm", bufs=1) as smpool,
            tc.tile_pool(name="ps", bufs=2, space="PSUM") as pp,
        ):
            # resident small tensors
            wa_t = [wpool.tile([128, 2304], bf16, tag=f"wa{k}", name=f"wa{k}")
                    for k in range(2)]
            wb_t = [wpool.tile([128, 2304], bf16, tag=f"wb{k}", name=f"wb{k}")
                    for k in range(2)]
            wl_t = wpool.tile([128, 2 * NCLS], bf16, tag="wl", name="wl")
            bab_t = wpool.tile([128, 4], f32, tag="bab", name="bab")
            bl_t = wpool.tile([128, NCLS], f32, tag="bl", name="bl")
            mt_t = wpool.tile([128, 3 * WP], bf16, tag="mt", name="mt")
            mb_t = wpool.tile([128, 3 * WP], bf16, tag="mb", name="mb")
            for k in range(2):
                nc.sync.dma_start(wa_t[k][:], d_wa.ap()[k])
                nc.sync.dma_start(wb_t[k][:], d_wb.ap()[k])
            nc.sync.dma_start(wl_t[:], d_wl.ap())
            nc.sync.dma_start(bab_t[:], d_bab.ap())
            nc.sync.dma_start(bl_t[:], d_bl.ap())
            nc.sync.dma_start(mt_t[:], d_mt.ap())
            nc.sync.dma_start(mb_t[:], d_mb.ap())

            s1 = [bigpool.tile([128, S1_N], bf16, tag="big", name=f"s1_{k}")
                  for k in range(2)]
            a = [bigpool.tile([128, A_N], bf16, tag="big", name=f"a_{k}")
                 for k in range(2)]

            # zero s1 pad cols + boundary elems (never written by stage1)
            for k in range(2):
                v = s1[k][:, 1:1 + R_S1 * WP].rearrange(
                    "p (r w) -> p r w", w=WP)
                nc.vector.memset(v[:, :, 0:1], 0.0)
                nc.vector.memset(v[:, :, 129:130], 0.0)
                nc.vector.memset(s1[k][:, 0:1], 0.0)
                nc.vector.memset(s1[k][:, S1_N - 1:S1_N], 0.0)
                nc.vector.memset(a[k][:, 0:1], 0.0)
                nc.vector.memset(a[k][:, A_N - 1:A_N], 0.0)

            # ---- stage 1: fused upsample + smooth1
            for i in range(R_S1):
                xt = xpool.tile([128, C], bf16, tag="xt", name="xt")
                nc.sync.dma_start(xt[:], d_xrep.ap()[i])
                st = s2pool.tile([128, WL], bf16, tag="st", name="st")
                nc.sync.dma_start(st[:], d_s2m.ap()[i])
                for m in range(2):
                    ps = pp.tile([128, 128], f32, tag="s1p", name="s1p")
                    nc.tensor.matmul(ps[:], xt[:, m * 128:(m + 1) * 128],
                                     st[:], start=True, stop=True)
                    dst = s1[m][:, 1 + i * WP + 1:1 + i * WP + 129]
                    if (2 * i + m) % 2 == 0:
                        nc.scalar.copy(dst, ps[:])
                    else:
                        nc.vector.tensor_copy(dst, ps[:])

            # ---- conv helper
            def conv(inp, w_t, out_t, n_out, bias_col0):
                npix = n_out * WP
                nblk = (npix + 511) // 512
                for m in range(2):
                    for nb in range(nblk):
                        q0 = nb * 512
                        bs = min(512, npix - q0)
                        ps = pp.tile([128, 512], f32, tag="cp", name="cp")
                        idx = 0
                        for kt in range(2):
                            for di in range(3):
                                for dj in range(3):
                                    off = q0 + di * WP + dj
                                    nc.tensor.matmul(
                                        ps[:, :bs],
                                        w_t[kt][:, (m * 9 + di * 3 + dj) * 128:
                                                (m * 9 + di * 3 + dj) * 128 + 128],
                                        inp[kt][:, off:off + bs],
                                        start=(idx == 0), stop=(idx == 17))
                                    idx += 1
                        nc.scalar.activation(
                            out_t[m][:, 1 + q0:1 + q0 + bs], ps[:, :bs], Relu,
                            bias=bab_t[:, bias_col0 + m:bias_col0 + m + 1])

            # ---- conv_a, then mask halo rows + re-zero pad cols
            conv(s1, wa_t, a, R_A, 0)
            for m in range(2):
                nc.vector.tensor_mul(a[m][:, 1:1 + 3 * WP],
                                     a[m][:, 1:1 + 3 * WP], mt_t[:])
                o = 1 + (R_A - 3) * WP
                nc.vector.tensor_mul(a[m][:, o:o + 3 * WP],
                                     a[m][:, o:o + 3 * WP], mb_t[:])
                v = a[m][:, 1:1 + R_A * WP].rearrange("p (r w) -> p r w", w=WP)
                nc.vector.memset(v[:, :, 0:1], 0.0)
                nc.vector.memset(v[:, :, 129:130], 0.0)

            # ---- conv_b
            b2 = [bigpool.tile([128, B2_N], bf16, tag="big", name=f"b2_{k}")
                  for k in range(2)]
            conv(a, wb_t, b2, R_B2, 2)

            # ---- z = commuted 1x1 (pixel-major rows)
            z_pm = smpool.tile([128, R_B2 * NCLS], bf16, tag="z", name="z_pm")
            for k in range(R_B2):
                ps = pp.tile([128, NCLS], f32, tag="zp", name="zp")
                for kt in range(2):
                    nc.tensor.matmul(ps[:],
                                     b2[kt][:, 1 + k * WP + 1:1 + k * WP + 129],
                                     wl_t[:, kt * NCLS:(kt + 1) * NCLS],
                                     start=(kt == 0), stop=(kt == 1))
                nc.vector.tensor_copy(z_pm[:, k * NCLS:(k + 1) * NCLS], ps[:])

            # ---- smooth2 (banded matmuls) + bias
            out_pm = smpool.tile([128, R_OUT * NCLS], f32, tag="o", name="out_pm")
            for m in range(R_OUT):
                ps = pp.tile([128, NCLS], f32, tag="op", name="op")
                for di in range(5):
                    sbt = sbpool.tile([128, WL], bf16, tag="sbt", name="sbt")
                    nc.sync.dma_start(sbt[:], d_sbm.ap()[m][di])
                    nc.tensor.matmul(ps[:], sbt[:],
                                     z_pm[:, (m + di) * NCLS:(m + di + 1) * NCLS],
                                     start=(di == 0), stop=(di == 4))
                nc.vector.tensor_add(out_pm[:, m * NCLS:(m + 1) * NCLS],
                                     ps[:], bl_t[:])

            nc.sync.dma_start(d_out.ap(), out_pm[:])

    nc.compile()
    return nc


def _make_runner(nc):
    """Cached replacement for run_bass_kernel_spmd's axon path: build the
    jitted shard_map executable once, reuse across calls."""
    import jax
    from jax.experimental.shard_map import shard_map
    from jax.sharding import Mesh, PartitionSpec
    from concourse import bass2jax
    import concourse.mybir as mybir

    bass2jax.install_neuronx_cc_hook()
    partition_name = (nc.partition_id_tensor.name
                      if nc.partition_id_tensor else None)
    in_names, out_names, out_avals, out_shapes = [], [], [], []
    for alloc in nc.m.functions[0].allocations:
        if not isinstance(alloc, mybir.MemoryLocationSet):
            continue
        name = alloc.memorylocations[0].name
        if alloc.kind == "ExternalInput":
            if name != partition_name:
                in_names.append(name)
        elif alloc.kind == "ExternalOutput":
            out_names.append(name)
            shape = tuple(alloc.tensor_shape)
            dtype = mybir.dt.np(alloc.dtype)
            out_avals.append(jax.core.ShapedArray(shape, dtype))
            out_shapes.append((shape, dtype))
    n_params, n_outs = len(in_names), len(out_names)
    all_names = tuple(in_names + out_names
                      + ([partition_name] if partition_name else []))

    def _body(*args):
        operands = list(args)
        if partition_name is not None:
            operands.append(bass2jax.partition_id_tensor())
        return tuple(bass2jax._bass_exec_p.bind(
            *operands, out_avals=tuple(out_avals), in_names=all_names,
            out_names=tuple(out_names), lowering_input_output_aliases=(),
            sim_require_finite=True, sim_require_nnan=True, nc=nc))

    devices = jax.devices()[:N_CORES]
    mesh = Mesh(np.asarray(devices), ("core",))
    in_specs = (PartitionSpec("core"),) * (n_params + n_outs)
    out_specs = (PartitionSpec("core"),) * n_outs
    donate = tuple(range(n_params, n_params + n_outs))
    sharded = jax.jit(shard_map(_body, mesh=mesh, in_specs=in_specs,
                                out_specs=out_specs, check_rep=False),
                      donate_argnums=donate, keep_unused=True)

    def run(maps):
        concat_in = [np.concatenate([np.asarray(maps[c][n])
                                     for c in range(N_CORES)], axis=0)
                     for n in in_names]
        concat_zeros = [np.zeros((N_CORES * s[0], *s[1:]), dt)
                        for (s, dt) in out_shapes]
        outs = sharded(*concat_in, *concat_zeros)
        return [{name: np.asarray(outs[i]).reshape(
                    N_CORES, *out_shapes[i][0])[c]
                 for i, name in enumerate(out_names)}
                for c in range(N_CORES)]

    return run


def kernel(**inputs):
    if 'runner' not in _CACHE:
        nc = _build()
        _CACHE['runner'] = _make_runner(nc)
    maps = _make_inmaps(inputs)
    results = _CACHE['runner'](maps)

    out = np.zeros((B, NCLS, HL, WL), np.float32)
    for core in range(N_CORES):
        b, r0 = _core_ranges(core)
        out[b, :, r0:r0 + 64, :] = results[core]["out"].transpose(2, 1, 0)
    return out
